# revision 29
# baseline (speedup 1.0000x reference)
"""Trainium2 Bass kernel for nn_DeformableBlock (offset-conv -> deformable
conv v1 -> GroupNorm(32) -> ReLU), 8-core SPMD.

Sharding: core c -> (batch b = c//2, row-half h = c%2), rows [32h, 32h+32).
GroupNorm statistics are AllReduce'd across each (b,0)/(b,1) core pair.

Per-core algorithm (z-first formulation):
  z_k = x . W_k (pointwise matmul per 3x3 tap) over a 44-row window, stored
  bf16 in DRAM as y-pair rows ypt[j] = (z[j], z[j+64]); one dma_gather per
  tap with OVERLAPPING 2KB elements (elem_size=1024, elem_step=512) reads
  ypt rows j and j+1 per index = all four bilinear corners
  (y,x0),(y+1,x0),(y,x0+1),(y+1,x0+1) in one descriptor, then fused
  scalar_tensor_tensor accumulate on the vector engine.
Offsets come from a 3x3 conv done as im2col matmuls, PE-transposed to
position-major.  Gather indices are computed ON DEVICE directly in the
SWDGE wrapped-16 idx layout: offsets are re-broadcast into a replicated
(16a+v, s) layout (position p = 16s+v on every partition group a) with 8
host-constant permutation matmuls, and a second small DVE pipeline emits
int16 indices - no DRAM idx bounce.
"""
import functools
import numpy as np
import ml_dtypes

import concourse.bass as bass
import concourse.bacc as bacc
import concourse.mybir as mybir
import concourse.tile as tile
from concourse.bass_utils import run_bass_kernel_spmd

F32 = mybir.dt.float32
BF16 = mybir.dt.bfloat16
FP8 = mybir.dt.float8e4
I16 = mybir.dt.int16
I32 = mybir.dt.int32
AOP = mybir.AluOpType
ACT = mybir.ActivationFunctionType

B, CIN, COUT, H, W = 4, 256, 256, 64, 64
K = 9
WROWS = 44            # z window rows
XROWS = 35            # padded x slice rows (offset conv only; +1 slack row)
XCOLS = 66
NPOS = 2048           # output positions per core (32 rows)
NWIN = WROWS * 64     # z window positions
ZPAD = 72             # guard rows before the y-pair z table
NZROW = NWIN + 144
NT = 16               # output position tiles of 128
EPS = 1e-5
GN_N = 2 * NPOS * 8   # elements per GN group (both cores of the pair)

bf16 = ml_dtypes.bfloat16


def build_program(reps=1, use_cc=True):
    nc = bacc.Bacc(None, target_bir_lowering=False, num_devices=8)

    # ---------------- I/O ----------------
    xsl_d = nc.dram_tensor("xsl", [2, 128, XROWS, XCOLS], F32, kind="ExternalInput")
    xz_d = nc.dram_tensor("xz", [2, 128, NWIN], BF16, kind="ExternalInput")
    wdef_d = nc.dram_tensor("wdef", [2, 128, K, COUT], BF16, kind="ExternalInput")
    woff_d = nc.dram_tensor("woff", [2, 128, K, 18], F32, kind="ExternalInput")
    byc_d = nc.dram_tensor("byc", [128, NT, K], F32, kind="ExternalInput")
    bxc_d = nc.dram_tensor("bxc", [128, NT, K], F32, kind="ExternalInput")
    bycr_d = nc.dram_tensor("bycr", [128, K, 128], F32, kind="ExternalInput")
    bxcr_d = nc.dram_tensor("bxcr", [128, K, 128], F32, kind="ExternalInput")
    # per-core scalar replicated to [128,1]: y window-row bias (-10 - r0)
    wconst_d = nc.dram_tensor("wconst", [128, 1], F32, kind="ExternalInput")
    pmat_d = nc.dram_tensor("pmat", [8, 128, 128], F32, kind="ExternalInput")
    ident_d = nc.dram_tensor("ident", [128, 128], F32, kind="ExternalInput")
    onescol_d = nc.dram_tensor("onescol", [128, 1], F32, kind="ExternalInput")
    onesrow_d = nc.dram_tensor("onesrow", [1, 128], F32, kind="ExternalInput")
    gnab_d = nc.dram_tensor("gnab", [1, 512], F32, kind="ExternalInput")
    out_d = nc.dram_tensor("out", [NPOS, COUT], F32, kind="ExternalOutput")

    with tile.TileContext(nc) as tc:
        with (
            tc.tile_pool(name="const", bufs=1) as cpool,
            tc.tile_pool(name="wm", bufs=1) as wmpool,
            tc.tile_pool(name="zst", bufs=2) as zstpool,
            tc.tile_pool(name="g", bufs=1) as gpool,
            tc.tile_pool(name="acc", bufs=1) as accpool,
            tc.tile_pool(name="outp", bufs=1) as outpool,
            tc.tile_pool(name="ps", bufs=4, space="PSUM") as pspool,
            tc.tile_pool(name="ps2", bufs=1, space="PSUM") as ps2pool,
            tc.tile_pool(name="dram", bufs=1, space="DRAM") as dpool,
        ):
            # ---------------- load constants / inputs ----------------
            # z-gating loads first on sync; non-critical constants on scalar
            xz = cpool.tile([128, 2, NWIN], BF16, tag="xz", name="xz")
            for ci in range(2):
                nc.sync.dma_start(xz[:, ci], xz_d[ci])
            wdef = cpool.tile([128, 2, K, COUT], BF16, tag="wdef", name="wdef")
            for ci in range(2):
                nc.sync.dma_start(wdef[:, ci], wdef_d[ci])
            xsl = cpool.tile([128, 2, XROWS, XCOLS], F32, tag="xsl", name="xsl")
            for ci in range(2):
                nc.sync.dma_start(xsl[:, ci], xsl_d[ci])
            woff = cpool.tile([128, 2, K, 18], F32, tag="woff", name="woff")
            for ci in range(2):
                nc.sync.dma_start(woff[:, ci], woff_d[ci])
            byc = cpool.tile([128, NT, K], F32, tag="byc", name="byc")
            bxc = cpool.tile([128, NT, K], F32, tag="bxc", name="bxc")
            nc.scalar.dma_start(byc[:], byc_d[:])
            nc.scalar.dma_start(bxc[:], bxc_d[:])
            bycr = cpool.tile([128, K, 128], F32, tag="bycr", name="bycr")
            bxcr = cpool.tile([128, K, 128], F32, tag="bxcr", name="bxcr")
            nc.scalar.dma_start(bycr[:], bycr_d[:])
            nc.scalar.dma_start(bxcr[:], bxcr_d[:])
            wconst = cpool.tile([128, 1], F32, tag="wconst", name="wconst")
            nc.scalar.dma_start(wconst[:], wconst_d[:])
            pmat = cpool.tile([128, 8, 128], F32, tag="pmat", name="pmat")
            nc.scalar.dma_start(pmat[:], pmat_d[:].rearrange("u p m -> p u m"))
            ident = cpool.tile([128, 128], F32, tag="ident", name="ident")
            nc.scalar.dma_start(ident[:], ident_d[:])
            onescol = cpool.tile([128, 1], F32, tag="onescol", name="onescol")
            nc.scalar.dma_start(onescol[:], onescol_d[:])
            onesrow = cpool.tile([1, 128], F32, tag="onesrow", name="onesrow")
            nc.scalar.dma_start(onesrow[:], onesrow_d[:])
            gnab = cpool.tile([1, 512], F32, tag="gnab", name="gnab")
            nc.scalar.dma_start(gnab[:], gnab_d[:])

            zbuf = dpool.tile([K, NZROW, 2 * COUT], BF16, tag="zbuf", name="zbuf")
            ccin = dpool.tile([1, 64], F32, tag="ccin", name="ccin")
            ccout = dpool.tile([1, 64], F32, tag="ccout", name="ccout")

            # zero guard/boundary rows of every tap's zquad table: top
            # [0, ZPAD), bottom [ZPAD+NWIN-66, NZROW) - z stores overwrite
            # the live slots afterwards; stale quad slots stay zero.
            zguard = cpool.tile([128, 2 * COUT], BF16, tag="zg", name="zg")
            nc.vector.memset(zguard[:], 0)
            zb_ap = zbuf[:]
            # only gather-reachable guard rows need zeroing: idx range is
            # [68, 2890] (+margin); stores later overwrite the live slots.
            gb0 = ZPAD + NWIN - 66

            def zero_guards(ks):
                for k in ks:
                    for i, (base, nrow) in enumerate(((60, 12), (gb0, 74))):
                        wr = bass.AP(
                            zb_ap.tensor,
                            zb_ap.offset + (k * NZROW + base) * 2 * COUT,
                            [[2 * COUT, nrow], [1, 2 * COUT]])
                        (nc.sync if i == 0 else nc.scalar).dma_start(
                            wr, zguard[0:nrow, :])

            zero_guards(range(2))

            for _rep in range(reps):
                # z production, LDWEIGHTS-grouped: tiles outer, taps inner
                # (the same xz tile is the stationary operand for every tap).
                def z_block(kgroup):
                    for half in range(2):
                        zsts = {k: zstpool.tile([128, 11, COUT], BF16,
                                                tag=f"zst{i}", name=f"zst{k}")
                                for i, k in enumerate(kgroup)}
                        for tt in range(11):
                            t = 11 * half + tt
                            zpss = {k: pspool.tile([128, COUT], F32, bufs=2,
                                                   tag=f"zps{i}", name="zps")
                                    for i, k in enumerate(kgroup)}
                            for ci in range(2):
                                for k in kgroup:
                                    nc.tensor.matmul(
                                        zpss[k][:], xz[:, ci, 128 * t:128 * (t + 1)],
                                        wdef[:, ci, k, :],
                                        start=(ci == 0), stop=(ci == 1))
                            for k in kgroup:
                                nc.scalar.copy(zsts[k][:, tt, :], zpss[k][:])
                        for k in kgroup:
                            row0 = k * NZROW + ZPAD + half * 1408
                            engs = (nc.sync, nc.scalar)
                            for s, dlt in enumerate((0, 64)):
                                wrS = bass.AP(
                                    zb_ap.tensor,
                                    zb_ap.offset + (row0 - dlt) * 2 * COUT + s * COUT,
                                    [[2 * COUT, 128], [128 * 2 * COUT, 11], [1, COUT]])
                                engs[s].dma_start(wrS, zsts[k][:])

                z_block((0, 1))
                zero_guards(range(2, K))

                # ---------------- offset conv: [18, 2048] via im2col ----------
                off_sb = cpool.tile([18, NPOS], F32, tag="off_sb", name="off_sb")
                offt = cpool.tile([128, NT, 18], F32, tag="offt", name="offt")
                xsl_flat = xsl[:].rearrange("p c r x -> p c (r x)")
                for q in range(6):  # 6-row chunks of output rows (last is 2)
                    nrows = 6 if q < 5 else 2
                    span = nrows * XCOLS
                    ops = ps2pool.tile([18, 6 * XCOLS], F32, tag="offps", name="offps")
                    first = True
                    for k in range(K):
                        ky, kx = k // 3, k % 3
                        base = (6 * q + ky) * XCOLS + kx
                        nc.tensor.matmul(
                            ops[:, 0:span], woff[:, 0, k, :],
                            xsl_flat[:, 0, base:base + span],
                            start=first, stop=False)
                        first = False
                        nc.tensor.matmul(
                            ops[:, 0:span], woff[:, 1, k, :],
                            xsl_flat[:, 1, base:base + span],
                            start=False, stop=(k == K - 1))
                    nc.scalar.copy(
                        off_sb[:, 384 * q:384 * q + 64 * nrows]
                        .rearrange("p (r x) -> p r x", x=64),
                        ops[:, 0:span].rearrange("p (r x) -> p r x", x=XCOLS)[:, :, 0:64])
                    # PE-transpose this chunk's tiles to position-major
                    for t in range(3 * q, min(3 * q + 3, NT)):
                        tps = ps2pool.tile([128, 18], F32, tag="tps", name="tps")
                        nc.tensor.transpose(
                            tps[:], off_sb[:, 128 * t:128 * (t + 1)],
                            ident[0:18, 0:18])
                        nc.vector.tensor_copy(offt[:, t, :], tps[:])

                # replicate offsets into wrapped layout: offtr[16a+v, s, :] =
                # off(p = 16s+v) via permutation matmuls P_u (u = s%8)
                offtr = cpool.tile([128, 128, 18], F32, tag="offtr", name="offtr")
                for u in range(8):
                    rps = ps2pool.tile([128, NT, 18], F32, tag="offps", name="rps")
                    nc.tensor.matmul(
                        rps[:].rearrange("p t c -> p (t c)"), pmat[:, u, :],
                        offt[:].rearrange("p t c -> p (t c)"),
                        start=True, stop=True)
                    nc.scalar.copy(offtr[:, u:128:8, :], rps[:])

                def dev_floor(src, tag, shape=None):
                    pool_shape = shape or [128, NT, K]
                    ii = wmpool.tile(pool_shape, I32, tag="flr_i", name=tag + "i")
                    ff = wmpool.tile(pool_shape, F32, tag=tag + "f", name=tag + "f")
                    gt = wmpool.tile(pool_shape, F32, tag="flr_g", name=tag + "g")
                    nc.vector.tensor_copy(ii[:], src[:])        # fp32 -> int32
                    nc.vector.tensor_copy(ff[:], ii[:])         # int32 -> fp32
                    nc.vector.tensor_tensor(gt[:], ff[:], src[:], op=AOP.is_gt)
                    nc.vector.tensor_tensor(ff[:], ff[:], gt[:], op=AOP.subtract)
                    return ff

                # ---------------- gather indices (replicated layout) ---------
                def wr(tag):
                    return wmpool.tile([128, K, 128], F32, tag=tag, name=tag)

                pyr = wr("pyr"); pxr = wr("pxr")
                nc.vector.tensor_add(
                    pyr[:], offtr[:, :, 0:18:2].rearrange("p s k -> p k s"),
                    bycr[:])
                nc.vector.tensor_add(
                    pxr[:], offtr[:, :, 1:18:2].rearrange("p s k -> p k s"),
                    bxcr[:])
                y0r = dev_floor(pyr, "y0r", shape=[128, K, 128])
                x0r = dev_floor(pxr, "x0r", shape=[128, K, 128])
                rwp = wmpool.tile([128, K, 128], F32, tag="pyr", name="rwp")
                nc.vector.tensor_scalar_add(rwp[:], y0r[:], wconst[:, 0:1])
                rw0 = wmpool.tile([128, K, 128], F32, tag="pxr", name="rw0")
                nc.vector.tensor_scalar(rw0[:], rwp[:], 0.0, 43.0, op0=AOP.max, op1=AOP.min)
                # idx = rw*64 + (x0r - 16) + ZPAD  (zquad row units)
                idxf = wmpool.tile([128, K, 128], F32, tag="idxf", name="idxf")
                nc.vector.tensor_scalar(
                    rw0[:], rw0[:], 64.0, float(ZPAD - 16), op0=AOP.mult, op1=AOP.add)
                nc.vector.tensor_tensor(idxf[:], rw0[:], x0r[:], op=AOP.add)
                idx16 = wmpool.tile([128, K, 128], I16, tag="idx16", name="idx16")
                nc.vector.tensor_copy(
                    idx16[:].rearrange("p k s -> p (k s)"),
                    idxf[:].rearrange("p k s -> p (k s)"))

                # ---------------- bilinear weights (plain layout, DVE) ------
                def wm(tag):
                    return wmpool.tile([128, NT, K], F32, tag=tag, name=tag)

                py = wm("py"); px = wm("px")
                nc.vector.tensor_add(py[:], offt[:, :, 0:18:2], byc[:])
                nc.vector.tensor_add(px[:], offt[:, :, 1:18:2], bxc[:])

                y0 = dev_floor(py, "y0")
                x0 = dev_floor(px, "x0")
                ty = wm("ty"); tx = wm("tx")
                nc.vector.tensor_tensor(ty[:], py[:], y0[:], op=AOP.subtract)
                nc.vector.tensor_tensor(tx[:], px[:], x0[:], op=AOP.subtract)
                y1 = wm("y1"); x1 = wm("x1")
                nc.vector.tensor_scalar_add(y1[:], y0[:], 1.0)
                nc.vector.tensor_scalar_add(x1[:], x0[:], 1.0)

                # validity from the global clamp (lifted bounds [16, 79])
                wgt_t = cpool.tile([128, 36, NT], F32, tag="wgt", name="wgt")

                vys = []
                for (yy, vtag) in ((y0, "0"), (y1, "1")):
                    yg = wm("yg" + vtag); vy = wm("vy" + vtag)
                    nc.vector.tensor_scalar(yg[:], yy[:], 16.0, 79.0, op0=AOP.max, op1=AOP.min)
                    nc.vector.tensor_tensor(vy[:], yg[:], yy[:], op=AOP.is_equal)
                    vys.append(vy)
                vxs = []
                for (xx, vtag) in ((x0, "0"), (x1, "1")):
                    xg = wm("xg" + vtag); vx = wm("vx" + vtag)
                    nc.vector.tensor_scalar(xg[:], xx[:], 16.0, 79.0, op0=AOP.max, op1=AOP.min)
                    nc.vector.tensor_tensor(vx[:], xg[:], xx[:], op=AOP.is_equal)
                    vxs.append(vx)

                omty = wm("omty"); omtx = wm("omtx")
                nc.vector.tensor_scalar(omty[:], ty[:], -1.0, 1.0, op0=AOP.mult, op1=AOP.add)
                nc.vector.tensor_scalar(omtx[:], tx[:], -1.0, 1.0, op0=AOP.mult, op1=AOP.add)
                wy = []
                for i, frac in enumerate((omty, ty)):
                    wv = wm("wy" + str(i))
                    nc.vector.tensor_tensor(wv[:], frac[:], vys[i][:], op=AOP.mult)
                    wy.append(wv)
                wx = []
                for i, frac in enumerate((omtx, tx)):
                    wv = wm("wx" + str(i))
                    nc.vector.tensor_tensor(wv[:], frac[:], vxs[i][:], op=AOP.mult)
                    wx.append(wv)

                # corner weights, laid out [128, kj, t] (kj = k*4 + 2*jy + jx)
                for jy in range(2):
                    for jx in range(2):
                        j = 2 * jy + jx
                        nc.vector.tensor_tensor(
                            wgt_t[:, j:36:4, :].rearrange("p k t -> p t k"),
                            wy[jy][:], wx[jx][:], op=AOP.mult)

                # ---------------- z matmuls + store bf16 x-quads ---------------
                z_block((2, 3))
                z_block((4, 5))
                z_block((6, 7))
                z_block((8,))

                # ---------------- gather + weighted accumulate ----------------
                acc = accpool.tile([128, NT, COUT], F32, tag="acc", name="acc")
                psums = wmpool.tile([128, NT, 32], F32, tag="psums", name="psums")
                psqs = wmpool.tile([128, NT, 32], F32, tag="psqs", name="psqs")
                sqt = wmpool.tile([128, COUT], F32, tag="sqt", name="sqt")
                AX = mybir.AxisListType.X
                for k in range(K):
                    g = gpool.tile([128, NT, 4 * COUT], BF16, tag="g0", name="g0")
                    zk = bass.AP(zb_ap.tensor,
                                 zb_ap.offset + k * NZROW * 2 * COUT,
                                 [[2 * COUT, NZROW - 1], [1, 4 * COUT]])
                    for hh in range(2):  # num_idxs>1024 overflows SWDGE ring
                        nc.gpsimd.dma_gather(
                            out_ap=g[:, 8 * hh:8 * (hh + 1), :],
                            in_ap=zk,
                            idxs_ap=idx16[:, k, 64 * hh:64 * (hh + 1)],
                            num_idxs=NPOS // 2,
                            num_idxs_reg=NPOS // 2,
                            elem_size=4 * COUT,
                            elem_step=2 * COUT,
                        )
                    for t in range(NT):
                        for s in range(4):
                            j = (0, 2, 1, 3)[s]
                            first = (k == 0 and s == 0)
                            nc.vector.scalar_tensor_tensor(
                                acc[:, t, :],
                                g[:, t, s * COUT:(s + 1) * COUT],
                                wgt_t[:, 4 * k + j, t:t + 1],
                                g[:, t, 0:COUT] if first else acc[:, t, :],
                                op0=AOP.mult,
                                op1=AOP.bypass if first else AOP.add)
                        if k == K - 1:
                            # tile t is final: fold its GN stats in now
                            nc.vector.tensor_reduce(
                                psums[:, t, :],
                                acc[:, t, :].rearrange("p (g c) -> p g c", c=8),
                                axis=AX, op=AOP.add)
                            nc.vector.tensor_tensor(
                                sqt[:], acc[:, t, :], acc[:, t, :], op=AOP.mult)
                            nc.vector.tensor_reduce(
                                psqs[:, t, :],
                                sqt[:].rearrange("p (g c) -> p g c", c=8),
                                axis=AX, op=AOP.add)

                # ---------------- GroupNorm stats + AllReduce ----------------
                stats = wmpool.tile([128, 64], F32, tag="stats", name="stats")
                nc.vector.tensor_reduce(
                    stats[:, 0:32], psums[:].rearrange("p t g -> p g t"),
                    axis=AX, op=AOP.add)
                nc.vector.tensor_reduce(
                    stats[:, 32:64], psqs[:].rearrange("p t g -> p g t"),
                    axis=AX, op=AOP.add)
                sps = ps2pool.tile([1, 64], F32, tag="sps", name="sps")
                nc.tensor.matmul(sps[:], onescol[:], stats[:], start=True, stop=True)
                stat_row = wmpool.tile([1, 64], F32, tag="strow", name="strow")
                nc.vector.tensor_copy(stat_row[:], sps[:])
                nc.sync.dma_start(ccin[:], stat_row[:])
                if use_cc:
                    nc.gpsimd.collective_compute(
                        "AllReduce", AOP.add,
                        replica_groups=[[0, 1], [2, 3], [4, 5], [6, 7]],
                        ins=[ccin[:].opt()], outs=[ccout[:].opt()],
                    )
                else:
                    nc.sync.dma_start(ccout[:], ccin[:])
                allst = wmpool.tile([1, 64], F32, tag="allst", name="allst")
                nc.sync.dma_start(allst[:], ccout[:])

                # mu = S/n; var = Q/n - mu^2; A = gamma*rstd; B = beta - mu*A
                mu = wmpool.tile([1, 32], F32, tag="mu", name="mu")
                var = wmpool.tile([1, 32], F32, tag="var", name="var")
                rstd = wmpool.tile([1, 32], F32, tag="rstd", name="rstd")
                nc.vector.tensor_scalar_mul(mu[:], allst[:, 0:32], 1.0 / GN_N)
                nc.vector.tensor_scalar_mul(var[:], allst[:, 32:64], 1.0 / GN_N)
                nc.vector.tensor_tensor(rstd[:], mu[:], mu[:], op=AOP.mult)
                nc.vector.tensor_tensor(var[:], var[:], rstd[:], op=AOP.subtract)
                nc.vector.tensor_scalar_add(var[:], var[:], EPS)
                nc.scalar.activation(rstd[:], var[:], ACT.Sqrt, bias=0.0)
                nc.vector.reciprocal(rstd[:], rstd[:])
                abrow = wmpool.tile([1, 512], F32, tag="abrow", name="abrow")
                rrep = wmpool.tile([1, 512], F32, tag="rrep", name="rrep")
                for c in range(8):
                    nc.vector.tensor_copy(rrep[0:1, c:256:8], rstd[:])
                    nc.vector.tensor_copy(rrep[0:1, 256 + c:512:8], mu[:])
                nc.vector.tensor_tensor(
                    abrow[:, 0:256], rrep[:, 0:256], gnab[:, 0:256], op=AOP.mult)
                nc.vector.tensor_tensor(
                    abrow[:, 256:512], rrep[:, 256:512], abrow[:, 0:256], op=AOP.mult)
                nc.vector.tensor_tensor(
                    abrow[:, 256:512], gnab[:, 256:512], abrow[:, 256:512],
                    op=AOP.subtract)
                abps = ps2pool.tile([128, 512], F32, tag="abps", name="abps")
                nc.tensor.matmul(abps[:], onesrow[:], abrow[:], start=True, stop=True)
                abbc = cpool.tile([128, 512], F32, tag="abbc", name="abbc")
                nc.scalar.copy(abbc[:], abps[:])

                # ---------------- apply GN + ReLU, write out ----------------
                for t in range(NT):
                    ot = outpool.tile([128, COUT], F32, tag="ot", name="ot")
                    nc.vector.tensor_tensor(ot[:], acc[:, t, :], abbc[:, 0:256], op=AOP.mult)
                    nc.vector.tensor_tensor(ot[:], ot[:], abbc[:, 256:512], op=AOP.add)
                    nc.scalar.activation(ot[:], ot[:], ACT.Relu)
                    od_ap = out_d[:, :]
                    wro = bass.AP(od_ap.tensor, od_ap.offset + t * 128 * COUT,
                                  [[COUT, 128], [1, COUT]])
                    (nc.sync if t % 2 == 0 else nc.scalar).dma_start(wro, ot[:])

    nc.compile()
    return nc


@functools.lru_cache(maxsize=1)
def _program():
    return build_program()


def _prep_core(core, x, offw, offb, dw):
    b, h = core // 2, core % 2
    r0 = 32 * h
    w0 = r0 - 6

    xsl = np.zeros((2, 128, XROWS, XCOLS), np.float32)
    for i, r in enumerate(range(r0 - 1, r0 + XROWS - 1)):
        if 0 <= r < H:
            xsl[0, :, i, 1:65] = x[b, 0:128, r, :]
            xsl[1, :, i, 1:65] = x[b, 128:256, r, :]
    xzarr = np.zeros((2, 128, WROWS, 64), np.float32)
    for i, r in enumerate(range(w0, w0 + WROWS)):
        if 0 <= r < H:
            xzarr[0, :, i, :] = x[b, 0:128, r, :]
            xzarr[1, :, i, :] = x[b, 128:256, r, :]

    # weights: wdef[ci, c, k, o] = dw[o, ci*128+c, ky, kx]
    dwr = dw.reshape(COUT, CIN, K).transpose(1, 2, 0)     # [cin, k, o]
    wdef = np.ascontiguousarray(
        dwr.reshape(2, 128, K, COUT)).astype(np.float32)
    owr = offw.reshape(18, CIN, K).transpose(1, 2, 0)      # [cin, k, 18]
    woff = np.ascontiguousarray(
        owr.reshape(2, 128, K, 18)).astype(np.float32)

    pos = np.arange(NPOS)
    prow = r0 + pos // 64
    pcol = pos % 64
    ky = np.arange(K) // 3
    kx = np.arange(K) % 3
    # lifted (+16) base grids with offset bias folded in
    by = prow[:, None] - 1.0 + ky[None, :] + offb[0::2][None, :] + 16.0
    bx = pcol[:, None] - 1.0 + kx[None, :] + offb[1::2][None, :] + 16.0
    # plain layout: [NPOS, K] -> [128, NT, K] with position p at (p%128, p//128)
    byc = by.reshape(NT, 128, K).transpose(1, 0, 2).astype(np.float32)
    bxc = bx.reshape(NT, 128, K).transpose(1, 0, 2).astype(np.float32)
    # replicated-wrapped layout: (16a+v, k, s) -> position p = 16s + v
    byr = by.reshape(128, 16, K)    # [s, v, k]
    bxr = bx.reshape(128, 16, K)
    bycr = np.zeros((128, K, 128), np.float32)
    bxcr = np.zeros((128, K, 128), np.float32)
    for a in range(8):
        bycr[16 * a:16 * (a + 1)] = byr.transpose(1, 2, 0)
        bxcr[16 * a:16 * (a + 1)] = bxr.transpose(1, 2, 0)

    wconst = np.full((128, 1), float(-10 - r0), np.float32)

    return {
        "xsl": np.ascontiguousarray(xsl),
        "xz": np.ascontiguousarray(xzarr.reshape(2, 128, NWIN)).astype(bf16),
        "wdef": wdef.astype(bf16), "woff": woff,
        "byc": np.ascontiguousarray(byc), "bxc": np.ascontiguousarray(bxc),
        "bycr": np.ascontiguousarray(bycr), "bxcr": np.ascontiguousarray(bxcr),
        "wconst": wconst,
    }


def kernel(x, offset_w, offset_b, deform_w, gn_gamma, gn_beta):
    x = np.asarray(x, np.float32)
    offw = np.asarray(offset_w, np.float32)
    offb = np.asarray(offset_b, np.float32)
    dw = np.asarray(deform_w, np.float32)
    gamma = np.asarray(gn_gamma, np.float32)
    beta = np.asarray(gn_beta, np.float32)

    nc = _program()

    ident = np.eye(128, dtype=np.float32)
    onescol = np.ones((128, 1), np.float32)
    onesrow = np.ones((1, 128), np.float32)
    gnab = np.concatenate([gamma, beta]).reshape(1, 512).astype(np.float32)
    # pmat[u, q, m] = 1 iff q == 16u + (m % 16)
    pmat = np.zeros((8, 128, 128), np.float32)
    for u in range(8):
        for m in range(128):
            pmat[u, 16 * u + (m % 16), m] = 1.0

    in_maps = []
    for core in range(8):
        m = _prep_core(core, x, offw, offb, dw)
        m.update({"ident": ident, "onescol": onescol, "onesrow": onesrow,
                  "gnab": gnab, "pmat": pmat})
        in_maps.append(m)

    global _last_in_maps
    _last_in_maps = in_maps

    res = run_bass_kernel_spmd(nc, in_maps, core_ids=list(range(8)))

    out = np.zeros((B, COUT, H, W), np.float32)
    for core in range(8):
        b, h = core // 2, core % 2
        o = res.results[core]["out"]  # [2048, 256]
        out[b, :, 32 * h:32 * h + 32, :] = (
            o.reshape(32, 64, COUT).transpose(2, 0, 1))
    return out
